# revision 24
# baseline (speedup 1.0000x reference)
"""Causal self-attention (B=2, T=2048, D=1024, 16 heads) on 8 trn2 cores.

Sharding: data-parallel over batch (4 cores per batch element), tensor-parallel
over heads (4 heads per core). Each core computes qkv/attention/proj for its
4 heads and produces a partial [T, D] projection output; the host sums the 4
partials of each batch element.

Host-side prep per core: x[b] transposed to [D, T] (the PE contracts over the
partition dim, so x^T is needed as the streaming operand) and the relevant
w_qkv / w_proj column/row slices, all cast to bf16. The 1/sqrt(d_head) score
scale is folded into w_q and w_k (each gets d_head**-0.25).

Perf structure vs the original baseline (~268us -> ~151us on the For_i
amortized-wall proxy):
- weight DMAs interleaved per-k-chunk ahead of the matching xT chunk (and the
  first xT chunk split in two), so the first qkv matmul starts ~2us in
  instead of waiting ~17us for weights queued behind all of xT.
- qk_startup puts 8 qkv accumulation groups in flight across all psum banks
  so the PE tracks the incoming xT DMA stream.
- S^T computed in 512-wide tq slabs with the two heads of a partition-pair
  (heads 2p / 2p+1 live in partitions 0:64 / 64:128) emitted as adjacent
  matmuls: their stationaries occupy disjoint PE row-groups (tile_position
  auto-derived from base_partition), so the HW runs them concurrently.
- one exp per (pair, slab, tk-chunk) covering both heads' psum banks; matmul
  and exp extents trimmed to the causal region at 128-col granularity.
- PV accumulates 4 tq-blocks of one head in a single psum bank; softmax
  normalization is one reciprocal + one stride-0-broadcast tensor_mul per
  4-block group. (A v-stationary PV streaming P^T — fewer, bigger matmuls —
  measured ~50us WORSE on HW despite a similar sim cost; the many small
  65-col matmuls pipeline well and were kept.)
- causal diag-block masking and the v_aug ones-columns run on the idle Pool
  engine (gpsimd namespace); the post-exp tail's psum->sbuf copies run on
  the scalar engine once the exps have drained.
- pt slabs triple-buffered (bufs=3, ~179KB/partition SBUF) so exp work for
  the next slab-pair starts while two earlier slabs are still consumed.
"""

import numpy as np
import ml_dtypes

import concourse.bass as bass
import concourse.mybir as mybir
import concourse.tile as tile
from concourse import bacc
from concourse.bass_utils import run_bass_kernel_spmd
from concourse.masks import make_identity, make_upper_triangular

B, T, D = 2, 2048, 1024
NH, DH = 16, 64
HPC = 4  # heads per core
NCORES = 8
KT = D // 128  # 8 contraction chunks for qkv matmuls
NT = T // 128  # 16 sequence chunks
SLAB = 512  # tq columns per attention slab
NS = T // SLAB  # 4 slabs

BF16 = mybir.dt.bfloat16
F16 = mybir.dt.float16
F32 = mybir.dt.float32
EXP = mybir.ActivationFunctionType.Exp

TRACE = False
LAST_RESULTS = None
_NC_CACHE = {}


def _build_program(loop_n=None):
    nc = bacc.Bacc("TRN2", target_bir_lowering=False, debug=False, num_devices=NCORES)
    xT_d = nc.dram_tensor("xT", [D, T], BF16, kind="ExternalInput").ap()
    wqk_d = nc.dram_tensor("wqk", [D, 2 * HPC * DH], BF16, kind="ExternalInput").ap()
    wv_d = nc.dram_tensor("wv", [D, HPC * DH], BF16, kind="ExternalInput").ap()
    wpr_d = nc.dram_tensor("wpr", [HPC * DH, D], BF16, kind="ExternalInput").ap()
    out_d = nc.dram_tensor("out", [T, D], F16, kind="ExternalOutput").ap()

    with tile.TileContext(nc) as tc:
        if loop_n is None:
            _emit(nc, tc, xT_d, wqk_d, wv_d, wpr_d, out_d)
        else:
            hints = (
                mybir.EngineType.PE,
                mybir.EngineType.Activation,
                mybir.EngineType.DVE,
                mybir.EngineType.SP,
                mybir.EngineType.Pool,
            )
            with tc.For_i(0, loop_n, 1, hint_engines=hints):
                _emit(nc, tc, xT_d, wqk_d, wv_d, wpr_d, out_d)
    nc.compile()
    return nc


def _emit(nc, tc, xT_d, wqk_d, wv_d, wpr_d, out_d):
    with (
        tc.tile_pool(name="big", bufs=1) as big,
        tc.tile_pool(name="pt_pool", bufs=3) as pt_pool,
        tc.tile_pool(name="small", bufs=1) as small,
        tc.tile_pool(name="stage", bufs=3) as stage,
        tc.tile_pool(name="ps_mm", bufs=2, space="PSUM") as ps_mm,
        tc.tile_pool(name="ps_s", bufs=2, space="PSUM") as ps_s,
        tc.tile_pool(name="ps_y", bufs=2, space="PSUM") as ps_y,
    ):
        # ---- load inputs: per-k-chunk DMAs, weight chunk ahead of its xT
        # chunk so the first accumulation group can start at ~2us ----
        xT_s = big.tile([128, KT, T], BF16)
        wqk_s = big.tile([128, KT, 2 * HPC * DH], BF16)
        xT_r = xT_d.rearrange("(a p) t -> p a t", p=128)
        wqk_r = wqk_d.rearrange("(a p) n -> p a n", p=128)
        for t in range(KT):
            nc.sync.dma_start(out=wqk_s[:, t, :], in_=wqk_r[:, t, :])
            if t == 0:
                # halves so the first qk matmul starts ~1us earlier
                nc.sync.dma_start(out=xT_s[:, 0, 0:1024], in_=xT_r[:, 0, 0:1024])
                nc.sync.dma_start(out=xT_s[:, 0, 1024:T], in_=xT_r[:, 0, 1024:T])
            else:
                nc.sync.dma_start(out=xT_s[:, t, :], in_=xT_r[:, t, :])
        wv_s = big.tile([128, KT, HPC * DH], BF16)
        nc.sync.dma_start(out=wv_s, in_=wv_d.rearrange("(a p) n -> p a n", p=128))
        wpr_s = big.tile([128, 2, D], BF16)
        nc.sync.dma_start(out=wpr_s, in_=wpr_d.rearrange("(a p) n -> p a n", p=128))

        ident = small.tile([128, 128], BF16)
        make_identity(nc, ident)
        # Dummy exp so walrus's ACT table load (~2.7us) happens during the
        # input-DMA ramp instead of at the first real exp on the critical path.
        warm = small.tile([128, 1], F32)
        nc.vector.memset(warm, 0.0)
        nc.scalar.activation(warm, warm, EXP)
        # gemask[p, f] = 1.0 where f >= p: the valid (tq >= tk) part of the
        # diagonal 128x128 block of S^T.
        gemask = small.tile([128, 128], BF16)
        make_upper_triangular(nc, gemask, val=1.0, diag=True)

        # q^T / k^T in [d, T] layout: tile p holds heads 2*p (parts 0:64)
        # and 2*p+1 (parts 64:128).
        qT_s = big.tile([128, 2, T], BF16)
        kT_s = big.tile([128, 2, T], BF16)
        # v in natural [tk, d] layout, a ones-column per head for rowsums
        v_aug = big.tile([128, NT, 66 * HPC], BF16)
        ones_ap = bass.AP(
            tensor=v_aug.tensor,
            offset=v_aug.offset + DH,
            ap=[v_aug.ap[0], [66 * HPC, NT], [66, HPC], [1, 1]],
        )
        nc.gpsimd.memset(ones_ap, 1.0)
        y_all = big.tile([128, NT, HPC * DH], BF16)
        yT_s = big.tile([128, 2, T], BF16)

        def qk_group(m, n, ps):
            # qk^T = wqk.T @ xT -> rows [128m : 128m+128] of [512, T], cols
            # [512n : 512n+512], accumulated over the 8 k-chunks into ps.
            for t in range(KT):
                nc.tensor.matmul(
                    ps,
                    lhsT=wqk_s[:, t, 128 * m : 128 * (m + 1)],
                    rhs=xT_s[:, t, 512 * n : 512 * (n + 1)],
                    start=(t == 0),
                    stop=(t == KT - 1),
                )
            dst = qT_s if m < 2 else kT_s
            nc.vector.tensor_copy(dst[:, m % 2, 512 * n : 512 * (n + 1)], ps)

        def qk_block(m, nrange=None):
            for n in nrange if nrange is not None else range(T // 512):
                qk_group(m, n, ps_mm.tile([128, 512], F32, tag="mm", name="qkps"))

        def qk_startup():
            # Heads-01 q (m=0) and k (m=2) over all of T, with all 8
            # accumulation groups in flight across every psum bank so the PE
            # stays saturated while it chases the incoming xT chunk DMAs.
            # Banks are borrowed from the attention pools; their first real
            # users only WAR-wait on the cheap qk copies.
            for n in range(2):
                qk_group(0, n, ps_mm.tile([128, 512], F32, tag="mm", name="qs0"))
                qk_group(2, n, ps_mm.tile([128, 512], F32, tag="mm", name="qs2"))
            sa = ps_s.tile([128, 2, SLAB], F32, tag="s")
            sb = ps_s.tile([128, 2, SLAB], F32, tag="s")
            qk_group(0, 2, sa[:, 0, :])
            qk_group(2, 2, sa[:, 1, :])
            qk_group(0, 3, sb[:, 0, :])
            qk_group(2, 3, sb[:, 1, :])
            ya = ps_y.tile([128, 4, 128], F32, tag="y")
            yb = ps_y.tile([128, 4, 128], F32, tag="y")
            qk_group(1, 0, ya.rearrange("p a b -> p (a b)"))
            qk_group(3, 0, yb.rearrange("p a b -> p (a b)"))

        def v_block(jlist):
            # v = x @ wv -> [T, 256] natural, scattered into v_aug
            for j in jlist:
                ps = ps_mm.tile([128, HPC * DH], F32, tag="mm")
                for t in range(KT):
                    nc.tensor.matmul(
                        ps,
                        lhsT=xT_s[:, t, 128 * j : 128 * (j + 1)],
                        rhs=wv_s[:, t, :],
                        start=(t == 0),
                        stop=(t == KT - 1),
                    )
                nc.vector.tensor_copy(
                    v_aug[:, j, :].rearrange("p (h c) -> p h c", c=66)[:, :, 0:DH],
                    ps.rearrange("p (h c) -> p h c", c=DH),
                )

        def s_exp_pair(p, s4):
            # S^T[tk, tq-slab] for heads 2p (parts 0:64) and 2p+1 (64:128),
            # emitted as adjacent matmuls on disjoint PE row-groups so they
            # run concurrently. One exp per (i) covers both heads' banks.
            ni = 4 * s4 + 4
            pt = pt_pool.tile([128, NT, 2, SLAB], BF16, tag="pt")
            for i in range(ni):
                c_lo = max(0, 128 * i - SLAB * s4)
                w = SLAB - c_lo
                ps = ps_s.tile([128, 2, SLAB], F32, tag="s")
                for h2 in range(2):
                    base = 64 * h2
                    nc.tensor.matmul(
                        ps[:, h2, c_lo:SLAB],
                        lhsT=kT_s[base : base + 64, p, 128 * i : 128 * (i + 1)],
                        rhs=qT_s[base : base + 64, p, SLAB * s4 + c_lo : SLAB * (s4 + 1)],
                        start=True,
                        stop=True,
                    )
                nc.scalar.activation(
                    pt[:, i, :, c_lo:SLAB], ps[:, :, c_lo:SLAB], EXP
                )
            # zero the invalid (tq < tk) parts of the four diagonal 128x128
            # blocks, both heads, in one strided Pool op: block r of head h2
            # sits at pt[:, 4*s4+r, h2, 128r : 128r+128].
            dv = pt[:, 4 * s4 : 4 * s4 + 4, :, 0:128]
            diag = bass.AP(
                tensor=dv.tensor,
                offset=dv.offset,
                ap=[dv.ap[0], [2 * SLAB + 128, 4], [SLAB, 2], [1, 128]],
            )
            gm = bass.AP(
                tensor=gemask.tensor,
                offset=gemask.offset,
                ap=[gemask.ap[0], [0, 4], [0, 2], [1, 128]],
            )
            nc.gpsimd.tensor_mul(diag, diag, gm)
            return pt

        def pv(h, s4, pt):
            # y[tq, 0:64] = sum_tk P~[tq, tk] v[tk, :] for the 4 tq-blocks of
            # slab s4, all in one psum bank; col 64 of each 128-col group is
            # the softmax denominator (ones-column of v_aug).
            p2, h2 = h // 2, h % 2
            ps = ps_y.tile([128, 4, 128], F32, tag="y")
            for jl in range(4):
                j = 4 * s4 + jl
                for i in range(j + 1):
                    nc.tensor.matmul(
                        ps[:, jl, 0:65],
                        lhsT=pt[:, i, h2, 128 * jl : 128 * (jl + 1)],
                        rhs=v_aug[:, i, 66 * h : 66 * h + 65],
                        start=(i == 0),
                        stop=(i == j),
                    )
            rinv = stage.tile([128, 4, 1], F32, tag="rinv")
            nc.vector.reciprocal(rinv, ps[:, :, 64:65])
            rinv_b = bass.AP(
                tensor=rinv.tensor,
                offset=rinv.offset,
                ap=[rinv.ap[0], [1, 4], [0, DH]],
            )
            nc.vector.tensor_mul(
                y_all[:, 4 * s4 : 4 * s4 + 4, DH * h : DH * (h + 1)],
                ps[:, :, 0:DH],
                rinv_b,
            )

        def _copy(out, in_, sc):
            # late-phase psum->sbuf copies go to the scalar engine, which is
            # idle once the exps drain, so DVE stops gating the proj tail
            if sc:
                nc.scalar.copy(out, in_)
            else:
                nc.vector.tensor_copy(out, in_)

        def trans_j(jlist, sc=False):
            for j in jlist:
                for dm in range(2):
                    pst = ps_mm.tile([128, 128], BF16, tag="mm")
                    nc.tensor.transpose(
                        pst, y_all[:, j, 128 * dm : 128 * (dm + 1)], ident
                    )
                    _copy(yT_s[:, dm, 128 * j : 128 * (j + 1)], pst, sc)

        def proj_j(jlist, sc=False):
            for j in jlist:
                for n in range(2):
                    ps = ps_mm.tile([128, 512], F32, tag="mm")
                    for dm in range(2):
                        nc.tensor.matmul(
                            ps,
                            lhsT=yT_s[:, dm, 128 * j : 128 * (j + 1)],
                            rhs=wpr_s[:, dm, 512 * n : 512 * (n + 1)],
                            start=(dm == 0),
                            stop=(dm == 1),
                        )
                    ost = stage.tile([128, 512], F16, tag="ost")
                    _copy(ost, ps, sc)
                    nc.sync.dma_start(
                        out=out_d[128 * j : 128 * (j + 1), 512 * n : 512 * (n + 1)],
                        in_=ost,
                    )

        def trans_proj(jlist, sc=False):
            # All transposes first, then all projs, so the per-j
            # PE->DVE->PE->DVE chains pipeline instead of serializing.
            trans_j(jlist, sc)
            proj_j(jlist, sc)

        # Emission order: get exp work onto ACT as early as possible, keep pt
        # slab liveness <= 2 (pool bufs), and interleave projection of ready
        # tq-blocks under later slabs' attention.
        qk_startup()
        pt00 = s_exp_pair(0, 0)
        pt01 = s_exp_pair(0, 1)
        pt02 = s_exp_pair(0, 2)
        v_block(range(8))
        pv(0, 0, pt00)
        pv(1, 0, pt00)
        pt03 = s_exp_pair(0, 3)
        qk_block(1, (1,))
        qk_block(3, (1,))
        pv(0, 1, pt01)
        pv(1, 1, pt01)
        pt10 = s_exp_pair(1, 0)
        v_block(range(8, NT))
        pv(0, 2, pt02)
        pv(1, 2, pt02)
        qk_block(1, (2, 3))
        qk_block(3, (2, 3))
        pt11 = s_exp_pair(1, 1)
        pv(0, 3, pt03)
        pv(1, 3, pt03)
        pt12 = s_exp_pair(1, 2)
        pv(2, 0, pt10)
        pv(3, 0, pt10)
        trans_proj(range(0, 4))
        pv(2, 1, pt11)
        pv(3, 1, pt11)
        pt13 = s_exp_pair(1, 3)
        trans_proj(range(4, 8))
        pv(2, 2, pt12)
        pv(3, 2, pt12)
        trans_proj(range(8, 12))
        pv(2, 3, pt13)
        pv(3, 3, pt13)
        for j in range(12, NT):
            trans_j([j], sc=True)
            proj_j([j], sc=True)


def _get_nc():
    if "nc" not in _NC_CACHE:
        _NC_CACHE["nc"] = _build_program()
    return _NC_CACHE["nc"]


def make_in_maps(x, w_qkv, w_proj):
    bf16 = ml_dtypes.bfloat16
    scale = np.float32(DH**-0.25)
    x = np.asarray(x, dtype=np.float32)
    w_qkv = np.asarray(w_qkv, dtype=np.float32)
    w_proj = np.asarray(w_proj, dtype=np.float32)
    xT_b = [np.ascontiguousarray(x[b].T).astype(bf16) for b in range(B)]
    in_maps = []
    for c in range(NCORES):
        b, g = c // HPC, c % HPC
        cs = slice(g * HPC * DH, (g + 1) * HPC * DH)  # 256 cols of this head group
        wq = w_qkv[:, 0 * D : 1 * D][:, cs] * scale
        wk = w_qkv[:, 1 * D : 2 * D][:, cs] * scale
        in_maps.append(
            {
                "xT": xT_b[b],
                "wqk": np.concatenate([wq, wk], axis=1).astype(bf16),
                "wv": np.ascontiguousarray(w_qkv[:, 2 * D : 3 * D][:, cs]).astype(bf16),
                "wpr": np.ascontiguousarray(w_proj[cs, :]).astype(bf16),
            }
        )
    return in_maps


def kernel(x, w_qkv, w_proj):
    global LAST_RESULTS
    nc = _get_nc()
    in_maps = make_in_maps(x, w_qkv, w_proj)
    res = run_bass_kernel_spmd(nc, in_maps, list(range(NCORES)), trace=TRACE)
    LAST_RESULTS = res
    parts = [np.asarray(res.results[c]["out"], dtype=np.float32) for c in range(NCORES)]
    out = np.stack([sum(parts[b * HPC : (b + 1) * HPC]) for b in range(B)], axis=0)
    return out.astype(np.float32)
